# revision 20
# baseline (speedup 1.0000x reference)
"""Trainium2 Bass kernel for nn_DiagonalTraining (anti-diagonal per-diag Linear).

out[b, r, c] = sum_{k} W[d, m, k] * x[b, r0(d)+k, d-r0(d)-k] + bias[d, m],
with d = r + c, m = r - r0(d).

Strategy: shard the 511 independent diagonals across 8 cores. All streams
are bf16 (rel-err budget 2e-2; measured ~2.4e-3), which halves HBM traffic
vs f32 and runs the PE at 1 cycle/row for any N.

Long diagonals (n > 128, d in [128,382], 255 of them) are grouped into
complementary PAIRS with nA + nB = 384 so the two chunk-1 k-ranges
(aA = nA-128, aB = nB-128, aA+aB = 128) exactly fill one 128-partition
stationary tile.  Per pair, 3 stationary xd tiles [128k x 128b] and 4 W
moving blocks:
  psum[:, 0:NA]    = xd0A.T @ W0A + xdp.T @ W1A   (chunk0 + chunk1 of A)
  psum[:, NA:NA+NB]= xd0B.T @ W0B + xdp.T @ W1B
W0A/W0B are sent at (near-)exact width; W1A/W1B carry structural zero rows
(the other pair member's partitions).

SPMD runs ONE program on 8 cores, so per-core column layouts must agree:
the 127 pairs + the standalone n=256 diagonal are sorted by size into 16
"slots"; slot u has one pair per core and a uniform (NA_u, NB_u) padded to
the slot max (pad <= 4 cols since sorted).

Short diagonals (n <= 128) keep the pair-packed block-diagonal bins of the
f32 baseline: 129 real bins + 7 dummies = 8 x 17, each one [128k x 128m]
matmul.

Outputs are written bf16, exact-packed, and unpacked/scattered on host.
Input DMAs issue from the Activation HWDGE ring, output DMAs from the SP
ring so the two streams interleave at the SDMA packet level.
"""

import sys

sys.path.insert(0, "/opt/trn_rl_repo")

import numpy as np

B, S = 128, 256
D = 2 * S - 1  # 511
NCORES = 8
NSLOT = 16  # long pair-slots per core
NSB = 17  # short bins per core
NPS = 8  # psum banks cycled over jobs

TRACE = False  # test.py sets True to pull exec_time_ns from the NTFF profile
last_results = None

# Input DMAs ride the two HWDGE rings (sync=SP, scalar=ACT) so both queue
# rows stream concurrently (~430 GB/s aggregate vs ~282 on one).  Early
# work ships in 3-slot groups (big DMAs stream best); the last 7 slots
# ship one DMA each so the tail isn't latency-quantized by a big group.
# ("G", u0, u1) = slots u0..u1-1, ("D",) = the shorts stream, ("L", u).
RING_A = [("G", 0, 3), ("D",), ("L", 10), ("L", 12), ("L", 14)]  # sync
RING_B = [("G", 3, 6), ("G", 6, 9), ("L", 9), ("L", 11), ("L", 13), ("L", 15)]
N_JOBS = 16 + NSB  # 33


def _geom(d):
    r0 = max(0, d - S + 1)
    n = d + 1 if d < S else 2 * S - 1 - d
    return r0, n


def _layout():
    """Global slot structure: slots[u][c] = (dA, dB|None), uniform shapes."""
    pairs = [(d, 382 - d) for d in range(128, 191)]  # left: nA+nB = 384
    pairs += [(d, 638 - d) for d in range(320, 383)]  # right (A = smaller n)
    pairs.append((191, 319))  # the two n=192 diagonals
    pairs.sort(key=lambda p: -_geom(p[0])[1])  # by nA desc
    rslots = [pairs[7 + 8 * u : 15 + 8 * u] for u in range(15)]
    # standalone slot: 7 biggest pairs on cores 0-6, the n=256 diag on core 7
    sx = pairs[:7] + [(255, None)]
    # job order ends with the smallest regular slots; the wide standalone
    # slot sits at index 12 so the tail stays small
    slots = rslots[:12] + [sx] + rslots[12:]

    shapes = []
    for ent in slots:
        NA = max(_geom(dA)[1] for dA, _ in ent)
        NB = max(_geom(dB)[1] if dB is not None else 0 for _, dB in ent)
        shapes.append((NA, NB))

    col0, CL = [], 0
    for NA, NB in shapes:
        col0.append(CL)
        CL += 384 + 2 * (NA + NB)
    ocol0, OL = [], 0
    for NA, NB in shapes:
        ocol0.append(OL)
        OL += NA + NB
    return slots, shapes, col0, CL, ocol0, OL


_SLOTS, _SHAPES, _COL0, CL, _OCOL0, OL = _layout()


def _short_bins():
    sbins = []
    for kk in range(1, 64):
        sbins.append([kk - 1, 127 - kk])
        sbins.append([511 - kk, 383 + kk])
    sbins.append([63, 447])
    sbins.append([127])
    sbins.append([383])
    sbins += [[] for _ in range(136 - len(sbins))]
    return sbins


def _wblk(d_, n_, koff, plo, phi, width):
    """W moving block [128, width]: [p, m] = W[d_, m, koff + p - plo]
    valid for p in [plo, phi) and m < n_; zero elsewhere."""
    p = np.arange(128)[:, None]
    m = np.arange(width)[None, :]
    kk = koff + (p - plo)
    msk = (p >= plo) & (p < phi) & (m < n_)
    idx = d_ * S * S + m * S + np.clip(kk, 0, S - 1)
    return np.where(msk, idx, 0).astype(np.int64), msk


def _diag_flat(d, kvals):
    """Flat x/grid index of diagonal d at positions kvals."""
    r0, n = _geom(d)
    r = r0 + kvals
    return r * S + (d - r)


def _core_tables():
    """Static per-core packing tables."""
    cores = []
    for c in range(NCORES):
        xdb = []  # (dstcol, idx[128], valid)
        wb = []  # (dstcol, idx[128, w], msk[128, w])
        tgt_l = np.full(OL, -1, np.int64)
        k = np.arange(128)
        for u in range(NSLOT):
            dA, dB = _SLOTS[u][c]
            NA, NB = _SHAPES[u]
            c0 = _COL0[u]
            r0A, nA = _geom(dA)
            aA = nA - 128
            xdb.append((c0, _diag_flat(dA, k), True))
            if dB is not None:
                r0B, nB = _geom(dB)
                xdb.append((c0 + 128, _diag_flat(dB, k), True))
            else:
                nB = 0
                xdb.append((c0 + 128, np.zeros(128, np.int64), False))
            # mixed chunk-1 stationary: p < aA -> A k=128+p, else B k=128+(p-aA)
            iA = _diag_flat(dA, np.minimum(128 + k, nA - 1))
            if dB is not None:
                iB = _diag_flat(dB, np.clip(128 + (k - aA), 0, nB - 1))
            else:
                iB = np.zeros(128, np.int64)
            xdb.append((c0 + 256, np.where(k < aA, iA, iB), True))
            # W moving blocks
            i0, m0 = _wblk(dA, nA, 0, 0, 128, NA)
            wb.append((c0 + 384, i0, m0))
            i1, m1 = _wblk(dB, nB, 0, 0, 128, NB) if dB is not None else (
                np.zeros((128, NB), np.int64), np.zeros((128, NB), bool))
            wb.append((c0 + 384 + NA, i1, m1))
            i2, m2 = _wblk(dA, nA, 128, 0, aA, NA)
            wb.append((c0 + 384 + NA + NB, i2, m2))
            i3, m3 = _wblk(dB, nB, 128, aA, 128, NB) if dB is not None else (
                np.zeros((128, NB), np.int64), np.zeros((128, NB), bool))
            wb.append((c0 + 384 + 2 * NA + NB, i3, m3))
            # output scatter targets
            tgt_l[_OCOL0[u] : _OCOL0[u] + nA] = _diag_flat(dA, np.arange(nA))
            if dB is not None:
                tgt_l[_OCOL0[u] + NA : _OCOL0[u] + NA + nB] = _diag_flat(
                    dB, np.arange(nB))

        # ---- short bins (same packing as the f32 baseline) ----
        sbins = _short_bins()
        my_s = sbins[c::NCORES]
        xds_i = np.zeros((NSB, 128), np.int64)
        xds_m = np.zeros((NSB, 128), np.float32)
        ws_i = np.zeros((NSB, 128, 128), np.int64)
        ws_m = np.zeros((NSB, 128, 128), np.float32)
        tgt_s = np.full((NSB, 128), -1, np.int64)
        for j, bin_ds in enumerate(my_s):
            off = 0
            for d in bin_ds:
                r0, n = _geom(d)
                i = np.arange(n)
                r = r0 + i
                col = d - r
                xds_i[j, off : off + n] = r * S + col
                xds_m[j, off : off + n] = 1.0
                ws_i[j, off : off + n, off : off + n] = (
                    d * S * S + i[None, :] * S + i[:, None]
                )
                ws_m[j, off : off + n, off : off + n] = 1.0
                tgt_s[j, off : off + n] = r * S + col
                off += n
        cores.append(
            dict(xdb=xdb, wb=wb, tgt_l=tgt_l, xds_i=xds_i, xds_m=xds_m,
                 ws_i=ws_i, ws_m=ws_m, tgt_s=tgt_s)
        )
    rr, cc = np.divmod(np.arange(S * S), S)
    dd = rr + cc
    r0v = np.maximum(0, dd - S + 1)
    bidx = dd * S + (rr - r0v)
    return cores, bidx


_TABLES = None
_PROG = None


def _tables():
    global _TABLES
    if _TABLES is None:
        _TABLES = _core_tables()
    return _TABLES


def _jobs():
    """Unified job order (matches cross-ring arrival order)."""
    jobs = [("L", u) for u in range(9)]
    jobs += [("S", j) for j in range(NSB)]
    jobs += [("L", u) for u in range(9, 16)]
    return jobs


def _cnt(k, e):
    """#copies on engine e (0=DVE, 1=ACT) among jobs 0..k (alternating)."""
    return (k + 2 - e) // 2 if k >= 0 else 0


def _build_program():
    import concourse.bass as bass
    import concourse.mybir as mybir

    f32 = mybir.dt.float32
    bf16 = mybir.dt.bfloat16
    nc = bass.Bass()
    dl = nc.dram_tensor("dl", [128, CL], bf16, kind="ExternalInput")
    ds = nc.dram_tensor("ds", [128, NSB * 256], bf16, kind="ExternalInput")
    yl = nc.dram_tensor("yl", [128, OL], bf16, kind="ExternalOutput")
    ys = nc.dram_tensor("ys", [128, NSB * 128], bf16, kind="ExternalOutput")

    # staging (one tensor per input DMA -> no WAR deps)
    def _slot_cols(u):
        return 384 + 2 * sum(_SHAPES[u])

    BTG = [
        nc.alloc_sbuf_tensor(
            f"btg{g}", [128, _COL0[u1 - 1] + _slot_cols(u1 - 1) - _COL0[u0]], bf16
        ).ap()
        for g, (u0, u1) in enumerate([(0, 3), (3, 6), (6, 9)])
    ]
    BTL = {
        u: nc.alloc_sbuf_tensor(f"btl{u}", [128, _slot_cols(u)], bf16).ap()
        for u in range(9, NSLOT)
    }
    BTS = nc.alloc_sbuf_tensor("bts", [128, NSB * 256], bf16).ap()
    YL = nc.alloc_sbuf_tensor("YL", [128, OL], bf16).ap()
    YS = nc.alloc_sbuf_tensor("YS", [128, NSB * 128], bf16).ap()
    PS = [nc.alloc_psum_tensor(f"ps{i}", [128, 512], f32).ap() for i in range(NPS)]

    # one DIN sem per input DMA; slot/shorts -> sem resolved via _job_sem
    DING = [nc.alloc_semaphore(f"dg{g}") for g in range(3)]
    DINL = {u: nc.alloc_semaphore(f"dl{u}") for u in range(9, NSLOT)}
    DINS = nc.alloc_semaphore("dsm")
    P = nc.alloc_semaphore("P")
    CV = nc.alloc_semaphore("CV")  # DVE copy completions (even jobs)
    CA = nc.alloc_semaphore("CA")  # ACT copy completions (odd jobs)
    DO = nc.alloc_semaphore("DO")

    jobs = _jobs()

    def _job_sem(kind, idx):
        if kind == "S":
            return DINS
        if idx < 9:
            return DING[idx // 3]
        return DINL[idx]

    # (last-job-index, tensor, col range, ring) — early outs ride the SWDGE
    # ring (HWDGE rings are busy with inputs); tail outs ride the HWDGE rings
    out_events = [
        (5, "yl", 0, _OCOL0[6], "gpsimd"),
        (8, "yl", _OCOL0[6], _OCOL0[9], "gpsimd"),
        (8 + NSB, "ys", 0, NSB * 128, "gpsimd"),
        (NSB + 11, "yl", _OCOL0[9], _OCOL0[12], "sync"),
        (NSB + 13, "yl", _OCOL0[12], _OCOL0[14], "scalar"),
        (N_JOBS - 1, "yl", _OCOL0[14], OL, "sync"),
    ]

    def _in_dma(eng, item):
        if item[0] == "G":
            _, u0, u1 = item
            g = u0 // 3
            eng.dma_start(
                out=BTG[g][:],
                in_=dl[:, _COL0[u0] : _COL0[u1 - 1] + _slot_cols(u1 - 1)],
            ).then_inc(DING[g], 16)
        elif item[0] == "L":
            u = item[1]
            eng.dma_start(
                out=BTL[u][:], in_=dl[:, _COL0[u] : _COL0[u] + _slot_cols(u)]
            ).then_inc(DINL[u], 16)
        else:
            eng.dma_start(out=BTS[:], in_=ds[:, :]).then_inc(DINS, 16)

    def _out_dma(eng, ev):
        k, which, o0, o1, _ = ev
        eng.wait_ge(CV, _cnt(k, 0))
        eng.wait_ge(CA, _cnt(k, 1))
        t, st = (yl, YL) if which == "yl" else (ys, YS)
        eng.dma_start(out=t[:, o0:o1], in_=st[:, o0:o1]).then_inc(DO, 16)

    def _copy(eng, sem, ji, kind, idx):
        eng.wait_ge(P, ji + 1)
        ps = PS[ji % NPS]
        if kind == "L":
            NA, NB = _SHAPES[idx]
            o = _OCOL0[idx]
            if eng is nc.vector:
                cp = eng.tensor_copy(YL[:, o : o + NA + NB], ps[:, 0 : NA + NB])
            else:
                cp = eng.copy(YL[:, o : o + NA + NB], ps[:, 0 : NA + NB])
        else:
            dst = YS[:, idx * 128 : (idx + 1) * 128]
            if eng is nc.vector:
                cp = eng.tensor_copy(dst, ps[:, 0:128])
            else:
                cp = eng.copy(dst, ps[:, 0:128])
        cp.then_inc(sem, 1)

    with nc.Block(no_gpsimd_drain=True) as block:

        @block.sync
        def _(sync):
            for item in RING_A:
                _in_dma(sync, item)
            for ev in out_events:
                if ev[4] == "sync":
                    _out_dma(sync, ev)
            sync.wait_ge(DO, 16 * len(out_events))

        @block.gpsimd
        def _(gpsimd):
            # early output DMAs on the SWDGE ring (3rd concurrent queue row)
            for ev in out_events:
                if ev[4] == "gpsimd":
                    _out_dma(gpsimd, ev)

        @block.scalar
        def _(scalar):
            for item in RING_B:
                _in_dma(scalar, item)
            for ji, (kind, idx) in enumerate(jobs):
                if ji % 2 == 1:
                    _copy(nc.scalar, CA, ji, kind, idx)
                for ev in out_events:
                    if ev[4] == "scalar" and ev[0] == ji:
                        _out_dma(scalar, ev)

        @block.tensor
        def _(tensor):
            waited = set()
            for ji, (kind, idx) in enumerate(jobs):
                sem = _job_sem(kind, idx)
                if id(sem) not in waited:
                    tensor.wait_ge(sem, 16)
                    waited.add(id(sem))
                if ji >= NPS:
                    prev = ji - NPS
                    tensor.wait_ge(CV if prev % 2 == 0 else CA, _cnt(prev, prev % 2))
                ps = PS[ji % NPS]
                if kind == "L":
                    NA, NB = _SHAPES[idx]
                    if idx < 9:
                        bt = BTG[idx // 3]
                        o = _COL0[idx] - _COL0[(idx // 3) * 3]
                    else:
                        bt = BTL[idx]
                        o = 0
                    NT = NA + NB
                    xa = bt[:, o : o + 128]
                    xb = bt[:, o + 128 : o + 256]
                    xp = bt[:, o + 256 : o + 384]
                    wA0 = bt[:, o + 384 : o + 384 + NA]
                    wB0 = bt[:, o + 384 + NA : o + 384 + NT]
                    w1 = bt[:, o + 384 + NT : o + 384 + 2 * NT]
                    # W1A|W1B are column-adjacent: one moving pass covers both.
                    # It opens the accumulation group over the full [0:NT) so
                    # the chunk-0 passes accumulate into sub-ranges (a single
                    # group per bank — interleaved groups misaccumulate on HW)
                    nc.tensor.matmul(
                        ps[:, 0:NT], xp, w1, start=True, stop=False,
                        skip_group_check=True,
                    )
                    nc.tensor.matmul(
                        ps[:, 0:NA], xa, wA0, start=False, stop=False,
                        skip_group_check=True,
                    )
                    mm = nc.tensor.matmul(
                        ps[:, NA:NT], xb, wB0, start=False, stop=True,
                        skip_group_check=True,
                    )
                else:
                    o = idx * 256
                    mm = nc.tensor.matmul(
                        ps[:, 0:128],
                        BTS[:, o : o + 128],
                        BTS[:, o + 128 : o + 256],
                        start=True,
                        stop=True,
                    )
                mm.then_inc(P, 1)

        @block.vector
        def _(vector):
            for ji, (kind, idx) in enumerate(jobs):
                if ji % 2 == 0:
                    _copy(nc.vector, CV, ji, kind, idx)

    return nc


def _get_program():
    global _PROG
    if _PROG is None:
        _PROG = _build_program()
    return _PROG


def _pack_core(t, x_flat, W_flat, np_bf16):
    dl = np.zeros((128, CL), np.float32)
    for c0, idx, valid in t["xdb"]:
        if valid:
            dl[:, c0 : c0 + 128] = x_flat[:, idx].T
    for c0, idx, msk in t["wb"]:
        w = idx.shape[1]
        if w:
            dl[:, c0 : c0 + w] = W_flat[idx] * msk
    xds = x_flat[:, t["xds_i"]] * t["xds_m"]  # [B, NSB, 128]
    ws = W_flat[t["ws_i"]] * t["ws_m"]  # [NSB, 128k, 128m]
    dsb = np.zeros((128, NSB * 256), np.float32)
    dsb3 = dsb.reshape(128, NSB, 256)
    dsb3[:, :, 0:128] = xds.transpose(2, 1, 0)
    dsb3[:, :, 128:256] = ws.transpose(1, 0, 2)
    return {"dl": dl.astype(np_bf16), "ds": dsb.astype(np_bf16)}


def kernel(x, W, b):
    import ml_dtypes
    from concourse.bass_utils import run_bass_kernel_spmd

    x = np.asarray(x, np.float32)
    W = np.asarray(W, np.float32)
    b = np.asarray(b, np.float32)
    cores, bidx = _tables()
    x_flat = x.reshape(B, S * S)
    W_flat = W.reshape(-1)
    np_bf16 = ml_dtypes.bfloat16
    in_maps = [_pack_core(t, x_flat, W_flat, np_bf16) for t in cores]
    nc = _get_program()
    res = run_bass_kernel_spmd(nc, in_maps, core_ids=list(range(NCORES)), trace=TRACE)
    global last_results
    last_results = res
    out_flat = np.zeros((B, S * S), np.float32)
    for c, t in enumerate(cores):
        ylv = np.asarray(res.results[c]["yl"], np.float32).reshape(B, -1)
        fl = t["tgt_l"]
        vl = fl >= 0
        out_flat[:, fl[vl]] = ylv[:, vl]
        ysv = np.asarray(res.results[c]["ys"], np.float32).reshape(B, -1)
        fs = t["tgt_s"].reshape(-1)
        vs = fs >= 0
        out_flat[:, fs[vs]] = ysv[:, vs]
    out_flat += b.reshape(-1)[bidx][None, :]
    return out_flat.reshape(B, S, S)
